# revision 1
# baseline (speedup 1.0000x reference)
"""ConvCaps (conv capsules + dynamic routing) on 8 trn2 NeuronCores.

Sharding: data-parallel over (batch b=4) x (output-row half=2) -> 8 shards,
one per core. All routing math is pointwise in (b, oh, ow), so each shard is
fully independent: no collectives. Each shard receives the 9 padded input
rows its 4 output rows need (halo included), computes im2col patching,
the 4x4 pose "voting" einsum, and 3 dynamic-routing iterations locally.

Hardcoded problem shape: x (4,32,16,16,16) f32, a (4,32,16,16) f32 (unused
by the math, kept for interface fidelity), W_ij (288,32,4,4) f32.
Returns (p_out (4,32,16,8,8), a_out (4,32,8,8)).
"""

import numpy as np

EPS = 1e-5
Bc, Cc, K, STRIDE, PAD, P = 32, 32, 3, 2, 1, 4
PSIZE = P * P
ITERS = 3
BKK = Bc * K * K  # 288
OH = OW = 8
HLOC = 4  # output rows per shard (oh split in halves)

# im2col window indices (static)
_IDH = (np.arange(0, 2 * HLOC, STRIDE)[:, None] + np.arange(K)[None, :])  # (4,3) in 9 local padded rows
_IDW = (np.arange(0, 16, STRIDE)[:, None] + np.arange(K)[None, :])        # (8,3) in 18 padded cols

_PMAP_FN = None  # cached compiled pmap


def _shard_math(jnp, jax, xs, W):
    """One shard: xs (32,16,9,18) padded slice, W (288,32,4,4).
    Returns p (32,16,4,8), a (32,4,8)."""
    xp = xs[:, :, _IDH, :]            # (B, ps, 4, 3, 18)
    xp = xp[:, :, :, :, _IDW]         # (B, ps, 4, 3, 8, 3)
    xp = jnp.transpose(xp, (0, 3, 5, 1, 2, 4))   # (B, kh, kw, ps, oh, ow)
    xv = xp.reshape(BKK, P, P, HLOC, OW)         # (B', i, j, oh, ow)
    v = jnp.einsum("Bijhw,BCjk->BCikhw", xv, W)  # (B', C, i, k, oh, ow)
    u = v.reshape(BKK, Cc, PSIZE, HLOC, OW)

    def safe_norm(s, axis):
        return jnp.sqrt(jnp.sum(s * s, axis=axis, keepdims=True) + EPS)

    norm = safe_norm(u, 2)                        # (B',C,1,h,w)
    mx = jnp.max(norm, axis=0, keepdims=True)
    mn = jnp.min(norm, axis=0, keepdims=True)
    u = u / (mx - mn)

    r = jnp.zeros((BKK, Cc, 1, HLOC, OW), dtype=u.dtype)
    vout = None
    for i in range(ITERS):
        c = jax.nn.softmax(r, axis=1)
        s = jnp.sum(c * u, axis=0, keepdims=True)     # (1,C,ps,h,w)
        ns = safe_norm(s, 2)
        vout = (ns / (1.0 + ns)) * s / ns
        if i != ITERS - 1:
            r = r + jnp.sum(u * vout, axis=2, keepdims=True)
    a_out = safe_norm(vout, 2)                        # (1,C,1,h,w)
    return vout[0], a_out[0, :, 0]


def _shards_from_x(x):
    """x (4,32,16,16,16) -> (8,32,16,9,18): shard s=(2*b+half) gets padded
    rows [8*half : 8*half+9] (the halo its 4 output rows need)."""
    xpad = np.pad(x, ((0, 0), (0, 0), (0, 0), (PAD, PAD), (PAD, PAD)))
    shards = np.empty((8, Bc, PSIZE, 9, 18), dtype=x.dtype)
    for b in range(4):
        for half in range(2):
            r0 = 8 * half
            shards[2 * b + half] = xpad[b, :, :, r0:r0 + 9, :]
    return shards


def _gather(p_sh, a_sh):
    """(8,32,16,4,8),(8,32,4,8) -> full (4,32,16,8,8),(4,32,8,8)."""
    p = np.asarray(p_sh).reshape(4, 2, Cc, PSIZE, HLOC, OW)
    p = np.transpose(p, (0, 2, 3, 1, 4, 5)).reshape(4, Cc, PSIZE, OH, OW)
    a = np.asarray(a_sh).reshape(4, 2, Cc, HLOC, OW)
    a = np.transpose(a, (0, 2, 1, 3, 4)).reshape(4, Cc, OH, OW)
    return p.astype(np.float32), a.astype(np.float32)


def _run_devices(x, W_ij):
    global _PMAP_FN
    import jax
    import jax.numpy as jnp

    if _PMAP_FN is None:
        fn = lambda xs, W: _shard_math(jnp, jax, xs, W)
        _PMAP_FN = jax.pmap(fn, in_axes=(0, 0), devices=jax.devices()[:8])
    shards = _shards_from_x(x)
    Wrep = np.broadcast_to(W_ij, (8,) + W_ij.shape)
    p_sh, a_sh = _PMAP_FN(shards, Wrep)
    return _gather(np.asarray(p_sh), np.asarray(a_sh))


def _run_numpy(x, W_ij):
    shards = _shards_from_x(x)

    class _np_jax:  # minimal shims so _shard_math runs on numpy
        class nn:
            @staticmethod
            def softmax(r, axis):
                e = np.exp(r - np.max(r, axis=axis, keepdims=True))
                return e / np.sum(e, axis=axis, keepdims=True)

    np.transpose_ = np.transpose
    outs = [_shard_math(np, _np_jax, shards[s].astype(np.float32), W_ij)
            for s in range(8)]
    p_sh = np.stack([o[0] for o in outs])
    a_sh = np.stack([o[1] for o in outs])
    return _gather(p_sh, a_sh)


def kernel(x, a, W_ij):
    x = np.asarray(x, dtype=np.float32)
    W_ij = np.asarray(W_ij, dtype=np.float32)
    try:
        return _run_devices(x, W_ij)
    except Exception:
        return _run_numpy(x, W_ij)


# revision 4
# speedup vs baseline: 1.4834x; 1.4834x over previous
"""ConvCaps (conv capsules + dynamic routing) on 8 trn2 NeuronCores.

Sharding: data-parallel over (batch b=4) x (output-row half=2) -> 8 shards,
one per core. All routing math is pointwise in (b, oh, ow), so each shard is
fully independent: no collectives. Each shard receives the 9 padded input
rows its 4 output rows need (halo included), computes im2col patching,
the 4x4 pose "voting" einsum, and 3 dynamic-routing iterations locally.

Hardcoded problem shape: x (4,32,16,16,16) f32, a (4,32,16,16) f32 (unused
by the math, kept for interface fidelity), W_ij (288,32,4,4) f32.
Returns (p_out (4,32,16,8,8), a_out (4,32,8,8)).
"""

import numpy as np

EPS = 1e-5
Bc, Cc, K, STRIDE, PAD, P = 32, 32, 3, 2, 1, 4
PSIZE = P * P
ITERS = 3
BKK = Bc * K * K  # 288
OH = OW = 8
HLOC = 4  # output rows per shard (oh split in halves)

# im2col window indices (static)
_IDH = (np.arange(0, 2 * HLOC, STRIDE)[:, None] + np.arange(K)[None, :])  # (4,3) in 9 local padded rows
_IDW = (np.arange(0, 16, STRIDE)[:, None] + np.arange(K)[None, :])        # (8,3) in 18 padded cols

_PMAP_FN = None  # cached compiled pmap


def _shard_math(jnp, jax, xs, W):
    """One shard: xs (32,16,9,18) padded slice, W (288,32,4,4).
    Returns p (32,16,4,8), a (32,4,8)."""
    xp = xs[:, :, _IDH, :]            # (B, ps, 4, 3, 18)
    xp = xp[:, :, :, :, _IDW]         # (B, ps, 4, 3, 8, 3)
    xp = jnp.transpose(xp, (0, 3, 5, 1, 2, 4))   # (B, kh, kw, ps, oh, ow)
    xv = xp.reshape(BKK, P, P, HLOC, OW)         # (B', i, j, oh, ow)
    v = jnp.einsum("Bijhw,BCjk->BCikhw", xv, W)  # (B', C, i, k, oh, ow)
    u = v.reshape(BKK, Cc, PSIZE, HLOC, OW)

    def safe_norm(s, axis):
        return jnp.sqrt(jnp.sum(s * s, axis=axis, keepdims=True) + EPS)

    norm = safe_norm(u, 2)                        # (B',C,1,h,w)
    mx = jnp.max(norm, axis=0, keepdims=True)
    mn = jnp.min(norm, axis=0, keepdims=True)
    u = u / (mx - mn)

    r = jnp.zeros((BKK, Cc, 1, HLOC, OW), dtype=u.dtype)
    vout = None
    for i in range(ITERS):
        c = jax.nn.softmax(r, axis=1)
        s = jnp.sum(c * u, axis=0, keepdims=True)     # (1,C,ps,h,w)
        ns = safe_norm(s, 2)
        vout = (ns / (1.0 + ns)) * s / ns
        if i != ITERS - 1:
            r = r + jnp.sum(u * vout, axis=2, keepdims=True)
    a_out = safe_norm(vout, 2)                        # (1,C,1,h,w)
    # single fused output buffer -> one D2H gather RPC per shard
    return jnp.concatenate(
        [vout[0].reshape(-1), a_out[0, :, 0].reshape(-1)])


def _shards_from_x(x):
    """x (4,32,16,16,16) -> (8,32,16,9,18): shard s=(2*b+half) gets padded
    rows [8*half : 8*half+9] (the halo its 4 output rows need)."""
    xpad = np.pad(x, ((0, 0), (0, 0), (0, 0), (PAD, PAD), (PAD, PAD)))
    shards = np.empty((8, Bc, PSIZE, 9, 18), dtype=x.dtype)
    for b in range(4):
        for half in range(2):
            r0 = 8 * half
            shards[2 * b + half] = xpad[b, :, :, r0:r0 + 9, :]
    return shards


def _gather(p_sh, a_sh):
    """(8,32,16,4,8),(8,32,4,8) -> full (4,32,16,8,8),(4,32,8,8)."""
    p = np.asarray(p_sh).reshape(4, 2, Cc, PSIZE, HLOC, OW)
    p = np.transpose(p, (0, 2, 3, 1, 4, 5)).reshape(4, Cc, PSIZE, OH, OW)
    a = np.asarray(a_sh).reshape(4, 2, Cc, HLOC, OW)
    a = np.transpose(a, (0, 2, 1, 3, 4)).reshape(4, Cc, OH, OW)
    return p.astype(np.float32), a.astype(np.float32)


def _run_devices(x, W_ij):
    global _PMAP_FN
    import jax
    import jax.numpy as jnp

    if _PMAP_FN is None:
        fn = lambda xs, W: _shard_math(jnp, jax, xs, W)
        _PMAP_FN = jax.pmap(fn, in_axes=(0, 0), devices=jax.devices()[:8])
    shards = _shards_from_x(x)
    Wrep = np.broadcast_to(W_ij, (8,) + W_ij.shape)
    out = _PMAP_FN(shards, Wrep)               # (8, 16384+512)
    flat = jax.device_get(out)
    npsz = Cc * PSIZE * HLOC * OW
    p_sh = flat[:, :npsz].reshape(8, Cc, PSIZE, HLOC, OW)
    a_sh = flat[:, npsz:].reshape(8, Cc, HLOC, OW)
    return _gather(p_sh, a_sh)


def _run_numpy(x, W_ij):
    shards = _shards_from_x(x)

    class _np_jax:  # minimal shims so _shard_math runs on numpy
        class nn:
            @staticmethod
            def softmax(r, axis):
                e = np.exp(r - np.max(r, axis=axis, keepdims=True))
                return e / np.sum(e, axis=axis, keepdims=True)

    flat = np.stack([_shard_math(np, _np_jax, shards[s].astype(np.float32), W_ij)
                     for s in range(8)])
    npsz = Cc * PSIZE * HLOC * OW
    p_sh = flat[:, :npsz].reshape(8, Cc, PSIZE, HLOC, OW)
    a_sh = flat[:, npsz:].reshape(8, Cc, HLOC, OW)
    return _gather(p_sh, a_sh)


def kernel(x, a, W_ij):
    x = np.asarray(x, dtype=np.float32)
    W_ij = np.asarray(W_ij, dtype=np.float32)
    try:
        return _run_devices(x, W_ij)
    except Exception:
        return _run_numpy(x, W_ij)
